# revision 42
# baseline (speedup 1.0000x reference)
"""ObjCondensationLoss Trainium2 kernel (8 NeuronCores, data-parallel over hits).

Reference semantics (N=100000 hits, K=256 clusters, D=3):
  L_beta = sum(1-beta_ak)/K + (S_b/N_b)*sum(beta[bg])
  q_i    = atanh(beta_i)^2 + q_min
  q_ak   = max_i q_i*M_ik ; x_a = x[argmax] (per cluster)
  L_v    = (1/N) sum_i q_i * sum_k (M*d2 + (1-M)*relu(1-d2)) * q_ak

Key identities used:
  - q is monotonic in beta  => one segment-argmax of beta gives beta_ak, q_ak, x_a.
  - q_i*q_k*relu(1-d2) = relu(q_i*q_k*(1-d2)) = relu(h_i . w_k)  with
      h_i = [q x0, q x1, q x2, q|x|^2, q],  w_k = [2q_k xa, -q_k, q_k(1-|xa|^2)]
    so the (N,K) potential matrix is ONE 5-contraction matmul + relu (bf16).
  - member (attractive) correction:
      Lv*N = sum_ik relu(v) + T1 - sum_mem v - sum_mem relu(v)
      T1         = sum_k q_ak * segsum_k(q_i)          (from HS matmul col 4)
      sum_mem v  = sum_kc w_kc * HS_kc,  HS = segsum of h vectors (one-hot matmul)
      sum_mem relu(v) = per-tile masked extract of relu(v), via the STORED
      phase-A bf16 masks and tensor_tensor_reduce.

Perf structure (v4):
  - warmup AllReduce at program start: overlaps the CC cores' fixed init
    window and absorbs the cold ALGO_MESH_BEGIN (~11us) so the real
    AllGather begins warm.
  - pass A: one-hot masks split ~68% ACT (Square+Relu integer-indicator
    chain) / 32% DVE; DVE does the fused stt (mult,max) accumulate over
    PACKED vals  val = toint(beta*2^16-0.5)*128 + j  so the argmax
    (beta_q, j, p) decodes arithmetically -- no indirect-DMA recovery.
  - each core pre-gathers its own winner's x_a coords (rank*NLOC input) and
    the AllGather ships (beta_q, x0, x1, x2); the post-AG tournament
    propagates coords, so no post-AG gather exists.
  - ST=32 staging + 49 hts transposes run inside the AllGather dead zone.
  - phase 2: bf16 5-contraction pair matmuls; relu 44/5 ACT/DVE; member
    extract 30% Frobenius (PE ones-matmul) / 70% stt-accum (DVE), balancing
    PE~DVE~ACT at ~40us.
  - tail: partial mcol reduce at 80% of phase 2; lv/N and sbet/(K*NC)
    pre-scaled so only recip+mult+2 adds follow the final AllReduce.
"""

import numpy as np

N = 100000
K = 256
NC = 8
NLOC = N // NC          # 12500
P = 128
J = 98                  # P*J = 12544 padded local hits
NPAD = P * J
Q_MIN = 0.5
S_B = 1.0
BIG = np.float32(1 << 27)

_CACHE = {}

import os as _os
POOL_MAX = int(_os.environ.get("POOL_MAX", "0"))   # every POOL_MAXth j maxed on GPSIMD (0=off; TT illegal on Pool in this ISA)
BQ = int(_os.environ.get("BQ", "1"))               # j's per PSUM relu batch
ACT_MOD = int(_os.environ.get("ACT_MOD", "9"))     # of 5 relu batches: this many on ACT
ACT_CUT = int(_os.environ.get("ACT_CUT", "8"))     # batches b%ACT_MOD < ACT_CUT -> ACT
EXTRACT = _os.environ.get("EXTRACT", "stt")        # ttr | stt
AMASK_MOD = int(_os.environ.get("AMASK_MOD", "22"))  # pass-A one-hot masks:
AMASK_CUT = int(_os.environ.get("AMASK_CUT", "15"))  # j%MOD<CUT built on ACT
                                                     # (2-op Square+Relu chain)
FROB_MOD = int(_os.environ.get("FROB_MOD", "10"))   # ph2 member-extract pairs:
FROB_CUT = int(_os.environ.get("FROB_CUT", "3"))    # (j//2)%MOD<CUT -> Frobenius


def _build_nc():
    import concourse.bass as bass
    import concourse.bacc as bacc
    import concourse.mybir as mybir
    import concourse.tile as tile
    from concourse.masks import make_identity

    f32 = mybir.dt.float32
    i32 = mybir.dt.int32
    u32 = mybir.dt.uint32
    Alu = mybir.AluOpType
    Act = mybir.ActivationFunctionType
    Ax = mybir.AxisListType

    nc = bacc.Bacc()

    xs = nc.dram_tensor("xs", [P, 3 * J], f32, kind="ExternalInput")
    bs = nc.dram_tensor("bs", [P, J], f32, kind="ExternalInput")
    ys = nc.dram_tensor("ys", [P, J], i32, kind="ExternalInput")
    xf = nc.dram_tensor("xf", [N, 3], f32, kind="ExternalInput")
    rk = nc.dram_tensor("rk", [P, 1], f32, kind="ExternalInput")
    out_dr = nc.dram_tensor("out", [1], f32, kind="ExternalOutput")

    ag_in = nc.dram_tensor("ag_in", [4, K], f32)
    ag_out = nc.dram_tensor("ag_out", [4 * NC, K], f32, addr_space="Shared")
    ar_in = nc.dram_tensor("ar_in", [1, 4], f32)
    ar_out = nc.dram_tensor("ar_out", [1, 4], f32, addr_space="Shared")
    dm_in = nc.dram_tensor("dm_in", [1, 4], f32)
    dm_out = nc.dram_tensor("dm_out", [1, 4], f32, addr_space="Shared")
    wg_in = nc.dram_tensor("wg_in", [1, 4], f32)
    wg_out = nc.dram_tensor("wg_out", [NC, 4], f32, addr_space="Shared")
    RG = [list(range(NC))]

    from contextlib import ExitStack
    with tile.TileContext(nc) as tc, ExitStack() as es:
        cp = es.enter_context(tc.tile_pool(name="cp", bufs=1))   # persistent sbuf
        mk = es.enter_context(tc.tile_pool(name="mk", bufs=8))   # scratch masks
        tr = es.enter_context(tc.tile_pool(name="tr", bufs=4))   # trash outputs
        pv = es.enter_context(tc.tile_pool(name="pv", bufs=3, space="PSUM"))
        pt = es.enter_context(tc.tile_pool(name="pt", bufs=2, space="PSUM"))
        ph = es.enter_context(tc.tile_pool(name="ph", bufs=1, space="PSUM"))
        _body(nc, tc, locals(), mybir, bass, make_identity,
              f32, i32, u32, Alu, Act, Ax,
              xs, bs, ys, xf, rk, out_dr, ag_in, ag_out, ar_in, ar_out,
              dm_in, dm_out, wg_in, wg_out, RG,
              cp, mk, tr, pv, pt, ph)
    if not nc.is_finalized():
        nc.finalize()
    return nc


def _body(nc, tc, _loc, mybir, bass, make_identity,
          f32, i32, u32, Alu, Act, Ax,
          xs, bs, ys, xf, rk, out_dr, ag_in, ag_out, ar_in, ar_out,
          dm_in, dm_out, wg_in, wg_out, RG,
          cp, mk, tr, pv, pt, ph):
    V = nc.vector
    S = nc.scalar
    G = nc.gpsimd
    T = nc.tensor
    KT = K // P  # 2 cluster tiles
    bf16 = mybir.dt.bfloat16

    # ---------------- constants & input loads ----------------
    identg = cp.tile([P, P], f32)
    make_identity(nc, identg[:])
    ident = cp.tile([P, P], f32)          # DVE-owned copy: single-sem deps
    V.tensor_copy(ident[:], identg[:])
    ident_bf = cp.tile([P, P], bf16)      # for bf16 transposes
    V.tensor_copy(ident_bf[:], identg[:])
    ones_col = cp.tile([P, 1], f32)
    V.memset(ones_col[:], 1.0)
    # block-diag pair weights buffer (filled post-select; zeros are constant)
    wall2 = cp.tile([37, 2 * K], bf16)
    V.memset(wall2[:], 0.0)
    sgn = cp.tile([1, 12], f32)  # lv signs/N: [r,-mrel,T1,T1,-mv,-mv,0,0,0,0,-mcol,-mcol]
    SCL = float(np.float32(1.0 / N))
    V.memset(sgn[:], 0.0)
    V.memset(sgn[:, 0:1], SCL)
    V.memset(sgn[:, 1:2], -SCL)
    V.memset(sgn[:, 2:4], SCL)
    V.memset(sgn[:, 4:6], -SCL)
    V.memset(sgn[:, 10:12], -SCL)

    iotaC_i = cp.tile([P, K], i32)       # 0..255 (mask cols = y)
    G.iota(iotaC_i[:], pattern=[[1, K]], base=0, channel_multiplier=0)
    iotaC_bf = cp.tile([P, K], bf16)
    V.tensor_copy(iotaC_bf[:], iotaC_i[:])

    jw_i = cp.tile([P, J], i32)          # column index j
    G.iota(jw_i[:], pattern=[[1, J]], base=0, channel_multiplier=0)
    jf_all = cp.tile([P, J], f32)
    V.tensor_copy(jf_all[:], jw_i[:])

    # warmup collective: overlaps the CC cores' fixed init window (~60 us)
    # and absorbs the cold ALGO_MESH_BEGIN cost so the real AllGather starts
    # warm.  Runs entirely under pass A.
    dm_sb = cp.tile([1, 4], f32)
    G.memset(dm_sb[:], 0.0)
    nc.sync.dma_start(out=dm_in[:], in_=dm_sb[:])
    G.collective_compute("AllReduce", mybir.AluOpType.add,
                         replica_groups=RG, ins=[dm_in[:]], outs=[dm_out[:]])

    # beta/y first: the ACT mask chain and DVE max-acc depend on them, while
    # x is only needed for the h-vector staging -- don't queue it ahead.
    beta_sb = cp.tile([P, J], f32)
    nc.sync.dma_start(out=beta_sb[:], in_=bs[:])
    y_i = cp.tile([P, J], i32)
    nc.sync.dma_start(out=y_i[:], in_=ys[:])
    x_sb = cp.tile([P, 3 * J], f32)      # (p, j*3+d) interleaved
    nc.sync.dma_start(out=x_sb[:], in_=xs[:])
    y_f = cp.tile([P, J], f32)
    V.tensor_copy(y_f[:], y_i[:])
    yn = cp.tile([P, J], f32)            # -y, ACT-mask bias operand
    V.tensor_scalar(out=yn[:], in0=y_f[:], scalar1=-1.0, scalar2=None,
                    op0=Alu.mult)

    # ---------------- background stats (local) ----------------
    bgcol = cp.tile([P, 1], f32)
    tr98 = tr.tile([P, J], f32)
    V.scalar_tensor_tensor(out=tr98[:], in0=y_f[:], scalar=-1.0, in1=beta_sb[:],
                           op0=Alu.is_equal, op1=Alu.mult, accum_out=bgcol[:])
    nbcol = cp.tile([P, 1], f32)
    tr98b = tr.tile([P, J], f32)
    V.tensor_scalar(out=tr98b[:], in0=y_f[:], scalar1=-1.0, scalar2=None,
                    op0=Alu.is_equal, op1=Alu.add, accum_out=nbcol[:])

    # ---------------- q_i and staged hit vectors ----------------
    lnA = cp.tile([P, J], f32)
    S.activation(lnA[:], beta_sb[:], Act.Ln, bias=1.0, scale=1.0)
    lnB = cp.tile([P, J], f32)
    S.activation(lnB[:], beta_sb[:], Act.Ln, bias=1.0, scale=-1.0)
    ath = cp.tile([P, J], f32)
    V.tensor_tensor(out=ath[:], in0=lnA[:], in1=lnB[:], op=Alu.subtract)
    sq4 = cp.tile([P, J], f32)
    S.activation(sq4[:], ath[:], Act.Square, bias=0.0, scale=0.5)  # atanh^2
    q0 = cp.tile([P, J], f32)
    V.tensor_scalar(out=q0[:], in0=sq4[:], scalar1=Q_MIN, scalar2=None,
                    op0=Alu.add)
    validm = cp.tile([P, J], f32)        # y >= -1 (bg included, pads out)
    V.tensor_scalar(out=validm[:], in0=y_f[:], scalar1=-1.5, scalar2=None,
                    op0=Alu.is_gt)
    q_all = cp.tile([P, J], f32)
    V.tensor_tensor(out=q_all[:], in0=q0[:], in1=validm[:], op=Alu.mult)

    # packed argmax value: val = toint(beta*2^16 - 0.5)*128 + j, exact in f32
    # (< 2^23).  Pass A maxes val instead of beta, so the winner's (beta, j)
    # decode is pure arithmetic -- no indirect-DMA argmax recovery.
    bscal = cp.tile([P, J], f32)
    V.tensor_scalar(out=bscal[:], in0=beta_sb[:], scalar1=65536.0, scalar2=-0.5,
                    op0=Alu.mult, op1=Alu.add)
    b16i = cp.tile([P, J], i32)
    V.tensor_copy(b16i[:], bscal[:])
    b16f = cp.tile([P, J], f32)
    V.tensor_copy(b16f[:], b16i[:])
    val_all = cp.tile([P, J], f32)
    V.scalar_tensor_tensor(out=val_all[:], in0=b16f[:], scalar=128.0,
                           in1=jf_all[:], op0=Alu.mult, op1=Alu.add)

    x0 = cp.tile([P, J], f32)
    x1 = cp.tile([P, J], f32)
    x2 = cp.tile([P, J], f32)
    V.tensor_copy(x0[:], x_sb[:, 0:3 * J:3])
    V.tensor_copy(x1[:], x_sb[:, 1:3 * J:3])
    V.tensor_copy(x2[:], x_sb[:, 2:3 * J:3])
    sqn = cp.tile([P, J], f32)
    tmpb = cp.tile([P, J], f32)
    V.tensor_tensor(out=sqn[:], in0=x0[:], in1=x0[:], op=Alu.mult)
    V.tensor_tensor(out=tmpb[:], in0=x1[:], in1=x1[:], op=Alu.mult)
    V.tensor_tensor(out=sqn[:], in0=sqn[:], in1=tmpb[:], op=Alu.add)
    V.tensor_tensor(out=tmpb[:], in0=x2[:], in1=x2[:], op=Alu.mult)
    V.tensor_tensor(out=sqn[:], in0=sqn[:], in1=tmpb[:], op=Alu.add)

    # dense (P, 5J) bf16 h vectors: lhsT slices for the HST seg-sum matmul.
    # (The ST=32 staging layout + hts transposes are deferred to the
    # AllGather dead zone -- see below.)
    ST = 32  # stride per hit-vector: PE weight chunks must sit at base 0/32/64
    hd = cp.tile([P, 5 * J], bf16)
    V.tensor_tensor(out=hd[:, 0:5 * J:5], in0=x0[:], in1=q_all[:], op=Alu.mult)
    V.tensor_tensor(out=hd[:, 1:5 * J:5], in0=x1[:], in1=q_all[:], op=Alu.mult)
    V.tensor_tensor(out=hd[:, 2:5 * J:5], in0=x2[:], in1=q_all[:], op=Alu.mult)
    V.tensor_tensor(out=hd[:, 3:5 * J:5], in0=sqn[:], in1=q_all[:], op=Alu.mult)
    V.tensor_copy(hd[:, 4:5 * J:5], q_all[:])

    # ---------------- pass A: segment max of beta ----------------
    # bf16 one-hot mask (stored for phase-2 extract reuse) + fused stt
    # (mult beta, max) accumulate on DVE; bf16 seg-sum matmul on PE.
    accA = cp.tile([P, K], f32)
    accD = cp.tile([P, K], f32)
    V.memset(accA[:], 0.0)
    V.memset(accD[:], 0.0)
    HST = ph.tile([5, K], f32, tag="HST")
    m2s = []
    for j in range(J):
        m2 = cp.tile([P, K], bf16, name=f"m2_{j}")
        if (j % AMASK_MOD) < AMASK_CUT:
            # ACT path: d=(iota-y)^2 then m2=relu(1-d); exact 0/1 for ints.
            # Offloads mask builds to the otherwise-idle ACT engine.
            dA = tr.tile([P, K], bf16, tag="dA", name=f"dA_{j}")
            S.activation(dA[:], iotaC_bf[:], Act.Square,
                         bias=yn[:, j:j + 1], scale=1.0)
            S.activation(m2[:], dA[:], Act.Relu, bias=1.0, scale=-1.0)
        else:
            V.tensor_scalar(out=m2[:], in0=iotaC_bf[:],
                            scalar1=y_f[:, j:j + 1], scalar2=None,
                            op0=Alu.is_equal)
        m2s.append(m2)
        # segment-sum of hit vectors (accumulated in PSUM across all j)
        T.matmul(out=HST[:], lhsT=hd[:, 5 * j:5 * j + 5], rhs=m2[:],
                 start=(j == 0), stop=(j == J - 1), skip_group_check=True)
        acc = accA if j % 2 == 0 else accD
        V.scalar_tensor_tensor(out=acc[:], in0=m2[:],
                               scalar=val_all[:, j:j + 1], in1=acc[:],
                               op0=Alu.mult, op1=Alu.max)
    V.tensor_tensor(out=accA[:], in0=accA[:], in1=accD[:], op=Alu.max)

    # transpose cluster columns -> accT (cluster on partition); per-cluster
    # max of the packed vals, then width-2 arithmetic decode of (beta_q,j,p).
    vs2 = cp.tile([P, 2], f32)
    ps2 = cp.tile([P, 2], f32)
    for kt in range(KT):
        pT2 = pt.tile([P, P], f32, tag="pT")
        T.transpose(out=pT2[:], in_=accA[:, kt * P:(kt + 1) * P],
                    identity=ident[:])
        accTs = cp.tile([P, P], f32, tag=f"accTs{kt}")
        S.copy(accTs[:], pT2[:])
        top8 = cp.tile([P, 8], f32, tag=f"top8{kt}")
        V.max(top8[:], accTs[:])
        idx8 = cp.tile([P, 8], u32, tag=f"idx8{kt}")
        V.max_index(idx8[:], top8[:], accTs[:])
        V.tensor_copy(ps2[:, kt:kt + 1], idx8[:, 0:1])
        V.tensor_copy(vs2[:, kt:kt + 1], top8[:, 0:1])
    # floor(vs2/128) robust to either to-int rounding semantics
    t2 = cp.tile([P, 2], f32)
    V.tensor_scalar(out=t2[:], in0=vs2[:], scalar1=1.0 / 128.0,
                    scalar2=None, op0=Alu.mult)
    ti_i = cp.tile([P, 2], i32)
    V.tensor_copy(ti_i[:], t2[:])
    ti_f = cp.tile([P, 2], f32)
    V.tensor_copy(ti_f[:], ti_i[:])
    cgt = cp.tile([P, 2], f32)
    V.tensor_tensor(out=cgt[:], in0=ti_f[:], in1=t2[:], op=Alu.is_gt)
    V.tensor_tensor(out=ti_f[:], in0=ti_f[:], in1=cgt[:], op=Alu.subtract)
    jf2 = cp.tile([P, 2], f32)
    V.scalar_tensor_tensor(out=jf2[:], in0=ti_f[:], scalar=-128.0,
                           in1=vs2[:], op0=Alu.mult, op1=Alu.add)
    bl2 = cp.tile([P, 2], f32)
    V.tensor_scalar(out=bl2[:], in0=ti_f[:], scalar1=1.0 / 65536.0,
                    scalar2=None, op0=Alu.mult)
    gl2 = cp.tile([P, 2], f32)
    V.scalar_tensor_tensor(out=gl2[:], in0=ps2[:], scalar=float(J),
                           in1=jf2[:], op0=Alu.mult, op1=Alu.add)
    # gather THIS core's winner coords pre-AllGather (rk = rank*NLOC input),
    # so the AllGather ships (beta_q, x0, x1, x2) and no post-AG gather of
    # x_a is needed.
    rk_sb = cp.tile([P, 1], f32)
    nc.sync.dma_start(out=rk_sb[:], in_=rk[:])
    ggl2 = cp.tile([P, 2], f32)
    V.tensor_scalar(out=ggl2[:], in0=gl2[:], scalar1=rk_sb[:, 0:1],
                    scalar2=None, op0=Alu.add)
    gi2l = cp.tile([P, 2], i32)
    V.tensor_copy(gi2l[:], ggl2[:])
    xal2 = cp.tile([P, 6], f32)          # (p, kt*3+d)
    for kt in range(KT):
        G.indirect_dma_start(
            out=xal2[:, 3 * kt:3 * kt + 3], out_offset=None, in_=xf[:],
            in_offset=bass.IndirectOffsetOnAxis(ap=gi2l[:, kt:kt + 1], axis=0))

    # ship (beta_q, x0, x1, x2) per (cluster, core): sb8 col order (a c)
    sb8 = cp.tile([P, 8], f32)
    V.tensor_copy(sb8[:, 0:2], bl2[:])
    V.tensor_copy(sb8[:, 2:8].rearrange("p (d t) -> p d t", d=3),
                  xal2[:].rearrange("p (t d) -> p d t", t=2))
    nc.sync.dma_start(out=ag_in[:].rearrange("a (c p) -> p (a c)", p=P),
                      in_=sb8[:])
    G.collective_compute("AllGather", mybir.AluOpType.bypass,
                         replica_groups=RG, ins=[ag_in[:]], outs=[ag_out[:]])

    # ---- AllGather dead zone: stage hit vectors + transposes (PE idle) ----
    staging = cp.tile([P, ST * J], bf16)  # (p, j*32+c): [qx0,qx1,qx2,q|x|^2,q,...]
    V.memset(staging[:], 0.0)
    for c in range(5):
        V.tensor_copy(staging[:, c:ST * J:ST], hd[:, c:5 * J:5])
    groups = [(2 * g, 2) for g in range(49)]
    hts = []
    for g0, gn in groups:
        pT = pt.tile([ST * gn, P], bf16, tag="pT")
        T.transpose(out=pT[:], in_=staging[:, ST * g0:ST * (g0 + gn)],
                    identity=ident_bf[:])
        hT = cp.tile([ST * gn, P], bf16, tag=f"hT{g0}")
        S.copy(hT[:], pT[:])
        hts.append(hT)

    bsrc = cp.tile([NC, K], f32)
    nc.sync.dma_start(out=bsrc[:], in_=ag_out[0:4 * NC:4, :])
    xg = []
    for d in range(3):
        xgd = cp.tile([NC, K], f32, name=f"xg{d}")
        nc.sync.dma_start(out=xgd[:], in_=ag_out[1 + d:4 * NC:4, :])
        xg.append(xgd)

    # global select, both cluster tiles fused into width-2 ops; tournament
    # propagates the winner's (x0,x1,x2) payload alongside beta.
    btw = cp.tile([P, 2 * NC], f32)
    xtw = [cp.tile([P, 2 * NC], f32, name=f"xtw{d}") for d in range(3)]
    for kt in range(KT):
        pT2 = pt.tile([P, NC], f32, tag="pT", name=f"pT2_{kt}")
        T.transpose(out=pT2[:], in_=bsrc[:, kt * P:(kt + 1) * P],
                    identity=ident[0:NC, 0:NC])
        V.tensor_copy(btw[:, kt * NC:(kt + 1) * NC], pT2[:])
        for d in range(3):
            pT3 = pt.tile([P, NC], f32, tag="pT", name=f"pT3_{kt}_{d}")
            T.transpose(out=pT3[:], in_=xg[d][:, kt * P:(kt + 1) * P],
                        identity=ident[0:NC, 0:NC])
            V.tensor_copy(xtw[d][:, kt * NC:(kt + 1) * NC], pT3[:])
    w_cur, x_cur, width = btw, xtw, NC
    while width > 1:
        h = width // 2
        wv = w_cur[:].rearrange("p (t c) -> p t c", t=2)
        ge = cp.tile([P, 2 * h], f32, name=f"ge_{h}")
        V.tensor_tensor(out=ge[:].rearrange("p (t c) -> p t c", t=2),
                        in0=wv[:, :, 0:h], in1=wv[:, :, h:width], op=Alu.is_ge)
        bnew = cp.tile([P, 2 * h], f32, name=f"bn_{h}")
        V.tensor_tensor(out=bnew[:].rearrange("p (t c) -> p t c", t=2),
                        in0=wv[:, :, 0:h], in1=wv[:, :, h:width], op=Alu.max)
        xnew = []
        for d in range(3):
            xv = x_cur[d][:].rearrange("p (t c) -> p t c", t=2)
            xd_ = cp.tile([P, 2 * h], f32, name=f"xd_{h}_{d}")
            V.tensor_tensor(out=xd_[:].rearrange("p (t c) -> p t c", t=2),
                            in0=xv[:, :, 0:h], in1=xv[:, :, h:width],
                            op=Alu.subtract)
            V.tensor_tensor(out=xd_[:], in0=ge[:], in1=xd_[:], op=Alu.mult)
            xn = cp.tile([P, 2 * h], f32, name=f"xn_{h}_{d}")
            V.tensor_tensor(out=xn[:].rearrange("p (t c) -> p t c", t=2),
                            in0=xd_[:].rearrange("p (t c) -> p t c", t=2),
                            in1=xv[:, :, h:width], op=Alu.add)
            xnew.append(xn)
        w_cur, x_cur, width = bnew, xnew, h
    bg2 = w_cur                             # (P,2): global beta per tile
    em2 = cp.tile([P, 2], f32)
    V.tensor_scalar(out=em2[:], in0=bg2[:], scalar1=0.0, scalar2=None,
                    op0=Alu.is_equal)
    om2 = cp.tile([P, 2], f32)
    V.tensor_scalar(out=om2[:], in0=em2[:], scalar1=-1.0, scalar2=1.0,
                    op0=Alu.mult, op1=Alu.add)
    # q_ak = (1-empty) * (atanh(beta_g)^2 + qmin), width-2
    la2 = cp.tile([P, 2], f32)
    S.activation(la2[:], bg2[:], Act.Ln, bias=1.0, scale=1.0)
    lb2 = cp.tile([P, 2], f32)
    S.activation(lb2[:], bg2[:], Act.Ln, bias=1.0, scale=-1.0)
    at2 = cp.tile([P, 2], f32)
    V.tensor_tensor(out=at2[:], in0=la2[:], in1=lb2[:], op=Alu.subtract)
    s42 = cp.tile([P, 2], f32)
    S.activation(s42[:], at2[:], Act.Square, bias=0.0, scale=0.5)
    qa2 = cp.tile([P, 2], f32)
    V.tensor_scalar(out=qa2[:], in0=s42[:], scalar1=Q_MIN, scalar2=None,
                    op0=Alu.add)
    V.tensor_tensor(out=qa2[:], in0=qa2[:], in1=om2[:], op=Alu.mult)
    beta_g = [bg2[:, 0:1], bg2[:, 1:2]]
    q_ak = [qa2[:, 0:1], qa2[:, 1:2]]
    # width-2 W build from the tournament's (P,2) coord channels
    sn2 = cp.tile([P, 2], f32)
    tsq = cp.tile([P, 2], f32)
    V.tensor_tensor(out=sn2[:], in0=x_cur[0][:], in1=x_cur[0][:], op=Alu.mult)
    V.tensor_tensor(out=tsq[:], in0=x_cur[1][:], in1=x_cur[1][:], op=Alu.mult)
    V.tensor_tensor(out=sn2[:], in0=sn2[:], in1=tsq[:], op=Alu.add)
    V.tensor_tensor(out=tsq[:], in0=x_cur[2][:], in1=x_cur[2][:], op=Alu.mult)
    V.tensor_tensor(out=sn2[:], in0=sn2[:], in1=tsq[:], op=Alu.add)
    q22 = cp.tile([P, 2], f32)
    V.tensor_scalar(out=q22[:], in0=qa2[:], scalar1=2.0, scalar2=None,
                    op0=Alu.mult)
    W2 = cp.tile([P, 10], f32)   # [kt0: 2q*xa,-q,q(1-|xa|^2) | kt1: ...]
    for d in range(3):
        V.tensor_tensor(out=W2[:, d:10:5], in0=x_cur[d][:], in1=q22[:],
                        op=Alu.mult)
    V.tensor_scalar(out=W2[:, 3:10:5], in0=qa2[:], scalar1=-1.0,
                    scalar2=None, op0=Alu.mult)
    t1m2 = cp.tile([P, 2], f32)
    V.tensor_scalar(out=t1m2[:], in0=sn2[:], scalar1=-1.0, scalar2=1.0,
                    op0=Alu.mult, op1=Alu.add)
    V.tensor_tensor(out=W2[:, 4:10:5], in0=t1m2[:], in1=qa2[:], op=Alu.mult)

    # Wall (5, 256): transposed cluster weights, both tiles side by side
    wallp = ph.tile([5, K], bf16, tag="wallp")
    wallt = cp.tile([P, 5 * KT], bf16)
    V.tensor_copy(wallt[:], W2[:])
    for kt in range(KT):
        T.transpose(out=wallp[:, kt * P:(kt + 1) * P],
                    in_=wallt[:, 5 * kt:5 * kt + 5],
                    identity=ident_bf[:])
    # pair weights: the first phase-2 matmul depends only on wall2
    V.tensor_copy(wall2[0:5, 0:K], wallp[:])
    V.tensor_copy(wall2[32:37, K:2 * K], wallp[:])

    # SUM assembly of everything already available (pre-phase-2, shortens tail)
    SUM = cp.tile([P, 12], f32)
    G.memset(SUM[:], 0.0)
    hs_sb = cp.tile([5, K], f32)
    S.copy(hs_sb[:], HST[:])
    for kt in range(KT):
        pT4 = pt.tile([P, 5], f32, tag="pT")
        T.transpose(out=pT4[:], in_=hs_sb[:, kt * P:(kt + 1) * P],
                    identity=ident[0:5, 0:5])
        hst_t = cp.tile([P, 5], f32, tag=f"hstt{kt}")
        S.copy(hst_t[:], pT4[:])
        # T1 col: q_ak * segsum_q
        V.tensor_tensor(out=SUM[:, 2 + kt:3 + kt], in0=q_ak[kt][:],
                        in1=hst_t[:, 4:5], op=Alu.mult)
        # sum_mem v col: dot(W_k, HS_k)
        wdot = cp.tile([P, 5], f32, tag=f"wdot{kt}")
        V.tensor_tensor(out=wdot[:], in0=W2[:, 5 * kt:5 * kt + 5],
                        in1=hst_t[:], op=Alu.mult)
        V.reduce_sum(out=SUM[:, 4 + kt:5 + kt], in_=wdot[:], axis=Ax.X)
        # sbet col: (1 - beta_g)/(K*NC).  beta_g is the GLOBAL max, identical
        # on every core, and this column rides the final AllReduce(add) --
        # so pre-divide by NC as well as K.
        V.tensor_scalar(out=SUM[:, 8 + kt:9 + kt], in0=beta_g[kt][:],
                        scalar1=-1.0 / (K * NC), scalar2=1.0 / (K * NC),
                        op0=Alu.mult, op1=Alu.add)
    V.tensor_copy(SUM[:, 6:7], bgcol[:])
    V.tensor_copy(SUM[:, 7:8], nbcol[:])

    # ---------------- phase 2: potential matmul ----------------
    # per unit (pair of j's or a single j): matmul(s) -> relu (ACT/DVE
    # split, rcol accum); member-relu total via Frobenius: t_j = m2_j *
    # ta_j (bf16 TT at 2x), column-summed by an accumulating ones-matmul.
    units = [(g, (2 * g, 2 * g + 1)) for g in range(49)]
    rcol = cp.tile([P, len(units)], f32)
    mcol = cp.tile([P, J], f32)
    G.memset(mcol[:], 0.0)
    ones_bf = cp.tile([P, 1], bf16)
    V.memset(ones_bf[:], 1.0)
    mrelP = ph.tile([1, 2 * K], f32, tag="mrelP")
    td4_cur = [None]
    # mrel ones-matmuls are emitted LAGGED so the in-order PE queue never
    # stalls waiting for the DVE mask*relu products.
    mrel_q = []
    mrel_n = [0]
    pend_td = [None]

    def flush_mrel(final=False):
        while mrel_q and (final or len(mrel_q) > 3):
            td4t, w4 = mrel_q.pop(0)
            T.matmul(out=mrelP[:, 0:w4 * K], lhsT=ones_bf[:],
                     rhs=td4t[:, 0:w4 * K],
                     start=(mrel_n[0] == 0), stop=(final and not mrel_q),
                     skip_group_check=True)
            mrel_n[0] += 1

    for u, (g, js) in enumerate(units):
        flush_mrel()
        if u == 40:
            # early partial mcol reduce (j<80 all extracted): shortens tail
            V.reduce_sum(out=SUM[:, 10:11], in_=mcol[:, 0:80], axis=Ax.X)
        W2 = len(js) * K
        pvt4 = pv.tile([P, W2], f32, tag="pvt4", name=f"pvt4_{u}")
        assert len(js) == 2
        T.matmul(out=pvt4[:], lhsT=hts[g][0:37, :], rhs=wall2[:],
                 start=True, stop=True, skip_group_check=True)
        ta = tr.tile([P, W2], bf16, tag="ta", name=f"ta_{u}")
        if u % ACT_MOD < ACT_CUT:
            S.activation(ta[:], pvt4[:], Act.Relu, accum_out=rcol[:, u:u + 1])
        else:
            V.tensor_scalar(out=ta[:], in0=pvt4[:], scalar1=0.0, scalar2=None,
                            op0=Alu.max, op1=Alu.add,
                            accum_out=rcol[:, u:u + 1])
        for q, j in enumerate(js):
            if ((j // 2) % FROB_MOD) < FROB_CUT:
                # Frobenius path: mask*relu product, column-summed by PE
                if j % 2 == 0:
                    td4_cur[0] = tr.tile([P, 2 * K], bf16, tag="td4",
                                         name=f"td4_{j}", bufs=6)
                td4 = td4_cur[0]
                V.tensor_tensor(out=td4[:, (j % 2) * K:(j % 2 + 1) * K],
                                in0=m2s[j][:],
                                in1=ta[:, q * K:(q + 1) * K], op=Alu.mult)
                if j % 2 == 1 or j == J - 1:
                    mrel_q.append((td4, (j % 2) + 1))
            else:
                # stt extract path: accumulate member relu into mcol
                td = tr.tile([P, K], bf16, tag="td", name=f"td_{j}")
                V.scalar_tensor_tensor(out=td[:], in0=iotaC_bf[:],
                                       scalar=y_f[:, j:j + 1],
                                       in1=ta[:, q * K:(q + 1) * K],
                                       op0=Alu.is_equal, op1=Alu.mult,
                                       accum_out=mcol[:, j:j + 1])
    flush_mrel(final=True)

    # ---------------- reductions & loss ----------------
    V.reduce_sum(out=SUM[:, 0:1], in_=rcol[:], axis=Ax.X)
    mrel_sb = cp.tile([1, 2 * K], f32)
    S.copy(mrel_sb[:], mrelP[:])
    V.reduce_sum(out=SUM[0:1, 1:2], in_=mrel_sb[:], axis=Ax.X)
    V.reduce_sum(out=SUM[:, 11:12], in_=mcol[:, 80:J], axis=Ax.X)

    SUMa = cp.tile([P, 12], f32)
    S.copy(SUMa[:], SUM[:])                # ACT launder: sum-matmul waits 1 sem
    sump = ph.tile([1, 12], f32, tag="wallp")
    T.matmul(out=sump[:], lhsT=ones_col[:], rhs=SUMa[:], start=True, stop=True)
    sums = cp.tile([1, 12], f32)
    V.tensor_copy(sums[:], sump[:])

    # lv_loc = r + T1(0) + T1(1) - mv0 - mv1 - mrel_frob - mrel_stt
    lvt = cp.tile([1, 12], f32)
    V.tensor_tensor(out=lvt[:], in0=sums[:], in1=sgn[:], op=Alu.mult)
    lv = cp.tile([1, 1], f32)
    V.reduce_sum(out=lv[:], in_=lvt[:], axis=Ax.X)

    # arp = [lv/N, bg_sum, nb_count, sbet/K] -- lv and sbet pre-scaled above
    arp = cp.tile([1, 4], f32)
    V.tensor_copy(arp[:, 0:1], lv[:])
    V.tensor_copy(arp[:, 1:2], sums[0:1, 6:7])
    V.tensor_copy(arp[:, 2:3], sums[0:1, 7:8])
    V.tensor_tensor(out=arp[:, 3:4], in0=sums[0:1, 8:9], in1=sums[0:1, 9:10],
                    op=Alu.add)
    nc.sync.dma_start(out=ar_in[:], in_=arp[:])
    G.collective_compute("AllReduce", mybir.AluOpType.add,
                         replica_groups=RG, ins=[ar_in[:]], outs=[ar_out[:]])
    ars = cp.tile([1, 4], f32)
    nc.sync.dma_start(out=ars[:], in_=ar_out[:])

    # loss = sbet/K + S_b*bg/nb + lv/N   (S_b == 1.0)
    nbinv = cp.tile([1, 1], f32)
    V.reciprocal(out=nbinv[:], in_=ars[:, 2:3])
    tbg = cp.tile([1, 1], f32)
    V.tensor_tensor(out=tbg[:], in0=ars[:, 1:2], in1=nbinv[:], op=Alu.mult)
    loss = cp.tile([1, 1], f32)
    V.tensor_tensor(out=loss[:], in0=ars[:, 3:4], in1=ars[:, 0:1], op=Alu.add)
    V.tensor_tensor(out=loss[:], in0=loss[:], in1=tbg[:], op=Alu.add)
    nc.sync.dma_start(out=out_dr[None, :], in_=loss[:])


def _shard_inputs(x, beta, y):
    x = np.ascontiguousarray(np.asarray(x, dtype=np.float32))
    beta = np.ascontiguousarray(np.asarray(beta, dtype=np.float32))
    y = np.ascontiguousarray(np.asarray(y)).astype(np.int32)
    in_maps = []
    for r in range(NC):
        sl = slice(r * NLOC, (r + 1) * NLOC)
        xp = np.zeros((NPAD, 3), np.float32)
        bp = np.zeros((NPAD,), np.float32)
        yp = np.full((NPAD,), -2, np.int32)
        xp[:NLOC] = x[sl]
        bp[:NLOC] = beta[sl]
        yp[:NLOC] = y[sl]
        in_maps.append({
            "xs": xp.reshape(P, 3 * J),
            "bs": bp.reshape(P, J),
            "ys": yp.reshape(P, J),
            "xf": x,
            "rk": np.full((P, 1), np.float32(r * NLOC), np.float32),
        })
    return in_maps


def _install_ntff_hook_shim():
    """antenv.axon_hooks is absent in this image; recreate it via ctypes
    so run_bass_kernel_spmd(trace=True) can capture NTFF profiles."""
    import sys
    import types
    try:
        import antenv.axon_hooks  # noqa: F401
        return
    except ImportError:
        pass
    try:
        import antenv
        from trn_agent_boot.trn_boot import _ntff_profile_via_ctypes
        hook = _ntff_profile_via_ctypes("/opt/axon/libaxon_pjrt.so")
        mod = types.ModuleType("antenv.axon_hooks")
        mod._hook = hook
        mod.get_axon_ntff_profile_hook = lambda: mod._hook
        mod.set_axon_ntff_profile_hook = lambda h: setattr(mod, "_hook", h)
        sys.modules["antenv.axon_hooks"] = mod
        antenv.axon_hooks = mod
    except Exception as e:  # degrade to no tracing
        print(f"ntff hook shim failed: {e}")


def kernel(x, beta, y, K=256, S_b=1.0, q_min=0.5):
    import os
    assert int(K) == 256 and float(S_b) == 1.0 and float(q_min) == 0.5
    if int(os.environ.get("KERNEL_TRACE", "0")):
        _install_ntff_hook_shim()
    if "nc" not in _CACHE:
        _CACHE["nc"] = _build_nc()
    from concourse.bass_utils import run_bass_kernel_spmd
    in_maps = _shard_inputs(x, beta, y)
    trace = bool(int(os.environ.get("KERNEL_TRACE", "0")))
    res = run_bass_kernel_spmd(_CACHE["nc"], in_maps, core_ids=list(range(NC)),
                               trace=trace)
    _CACHE["last_results"] = res
    return np.float32(np.asarray(res.results[0]["out"]).reshape(-1)[0])


def run_sim(x, beta, y):
    """Multi-core simulator run (no hardware)."""
    import concourse.bass_interp as bass_interp
    if "nc" not in _CACHE:
        _CACHE["nc"] = _build_nc()
    nc = _CACHE["nc"]
    in_maps = _shard_inputs(x, beta, y)
    sim = bass_interp.MultiCoreSim(nc, NC)
    for r in range(NC):
        for k, v in in_maps[r].items():
            sim.cores[r].tensor(k)[:] = v
    sim.simulate()
    return np.float32(np.asarray(sim.cores[0].mem_tensor("out")).reshape(-1)[0])


if __name__ == "__main__":
    import sys
    sys.path.insert(0, "/root/problem")
    import jax
    import reference
    with jax.default_device(jax.devices("cpu")[0]):
        inputs = reference.setup_inputs()
        inputs = {k: (np.asarray(v) if hasattr(v, "shape") else v)
                  for k, v in inputs.items()}
        expected = float(reference.reference(**inputs))
    if "--sim" in sys.argv:
        got = float(run_sim(inputs["x"], inputs["beta"], inputs["y"]))
    else:
        got = float(kernel(**{k: (np.asarray(v) if hasattr(v, "shape") else v)
                              for k, v in inputs.items()}))
    rel = abs(got - expected) / max(abs(expected), 1e-30)
    print(f"expected={expected!r} got={got!r} rel={rel:.3e}")

